# revision 16
# baseline (speedup 1.0000x reference)
"""CapsLayer2D Trainium2 kernel (8-core SPMD, data-parallel over batch).

Math: per position p (of B*R*C) and capsule n:
  U[n,i,o] = sum_e x[p,i,e] * W[n,i,e,o]          (u_hat)
  b0 = 1/64; 2x { v = squash(sum_i b*U); b += sum_o U*v }; out = squash(sum_i b*U)

Mapping:
  - 8 cores, 2 batches each -> 392 positions/core, 4 pos-blocks of 98.
  - Per block: u_hat via block-diagonal-W bf16 matmuls into PSUM, ACT-drained
    to bf16 SBUF; s0 = sum_i U/64 via dense K=1024 bf16 matmuls (v0 =
    squash(s0), exact since b0 is uniform); then 2 routing iterations on DVE.
  - Free-dim layout everywhere is (f, g, o, x) [n-pair f=5, K-chunk g=8,
    caps_dim o=16, x=(i8,n2)=16] so every DVE tensor_tensor has an innermost
    stride-1 bf16 run and hits the 2x_1P perf mode, with <=3 free AP dims
    (TENSOR3D ISA limit):
      * P-mul  U*vE (v pre-expanded over i8 by cheap 4x copies)
      * o-tree sums o-halves (innermost x=16)
      * Q-mul  b*U  broadcasts b over o (b's (i8,n2) is contiguous x=16)
      * gi-tree sums i8-halves (innermost n2) then g-halves (innermost 32)
  - Host pre-builds bf16 xT, BD(W), dense W with matching column orders and
    un-permutes the (f,o,n2) output columns back to (n,o).
"""
import numpy as np

import concourse.bacc as bacc
import concourse.bass as bass
import concourse.mybir as mybir
import concourse.tile as tile
from concourse.bass_utils import run_bass_kernel_spmd

N_CORES = 8
B, R, C = 16, 14, 14
N_IN, D_IN = 64, 16          # i, e
N_CAPS, CAPS_DIM = 10, 16    # n, o
IE = N_IN * D_IN             # 1024
POS = (B // N_CORES) * R * C # 392 positions per core
BLK = 98                     # pos-block size
NBLK = POS // BLK            # 4
NF = N_CAPS // 2             # 5 units of 2 capsules
NCH = IE // 128              # 8 contraction chunks
F32 = mybir.dt.float32
BF16 = mybir.dt.bfloat16


def _squash(nc, pool, s_ap, v_ap):
    """v = squash(s). s_ap f32 [98,160]=(f,o,n2) contiguous; v_ap [98,160]
    same layout (any dtype). Square runs on DVE (s*s) to avoid an ACT
    round-trip on the critical path; Sqrt stays on ACT."""
    P = s_ap.shape[0]
    sq = pool.tile([P, 160], F32, tag="sq")
    nc.vector.tensor_mul(sq[:], s_ap, s_ap)
    q = pool.tile([P, 10], F32, tag="q")
    # reduce over o: view (f, n2, o) with o innermost (stride 2)
    nc.vector.tensor_reduce(
        q[:].rearrange("p (f n) -> p f n", f=NF),
        sq[:].rearrange("p (f o n) -> p f n o", f=NF, o=16),
        axis=mybir.AxisListType.X, op=mybir.AluOpType.add)
    rt = pool.tile([P, 10], F32, tag="rt")
    nc.scalar.activation(rt[:], q[:], mybir.ActivationFunctionType.Sqrt)
    qp = pool.tile([P, 10], F32, tag="qp")
    nc.vector.tensor_scalar_add(qp[:], q[:], 1.0)
    rc = pool.tile([P, 10], F32, tag="rc")
    nc.vector.reciprocal(rc[:], qp[:])
    al = pool.tile([P, 10], F32, tag="al")
    nc.vector.tensor_mul(al[:], rt[:], rc[:])
    alb = al[:].rearrange("p (f n) -> p f n", f=NF) \
        .unsqueeze(2).broadcast_to([P, NF, 16, 2])
    nc.vector.tensor_mul(
        v_ap.rearrange("p (f o n) -> p f o n", f=NF, o=16),
        s_ap.rearrange("p (f o n) -> p f o n", f=NF, o=16), alb)


def build_kernel(dbg=False, repeat=1):
    nc = bacc.Bacc("TRN2", target_bir_lowering=False, debug=False,
                   num_devices=N_CORES)
    xT = nc.dram_tensor("xT", [IE, POS], BF16, kind="ExternalInput").ap()
    bdw = nc.dram_tensor("bdw", [128, NCH * NF * 256], BF16,
                         kind="ExternalInput").ap()
    wd = nc.dram_tensor("wd", [IE, NF * 32], BF16, kind="ExternalInput").ap()
    out = nc.dram_tensor("out", [POS, NF * 32], F32,
                         kind="ExternalOutput").ap()

    with tile.TileContext(nc) as tc:
        for _rep in range(repeat):
            with tc.tile_pool(name="const", bufs=1) as const, \
                 tc.tile_pool(name="work", bufs=3) as work:
                # warm the ACT tables (Sqrt/Copy) before the big DMAs so the
                # table-load DMAs don't queue behind them
                warm = const.tile([1, 2], F32)
                nc.vector.memset(warm[:], 1.0)
                nc.scalar.activation(warm[:], warm[:],
                                     mybir.ActivationFunctionType.Sqrt)
                nc.scalar.activation(warm[:], warm[:],
                                     mybir.ActivationFunctionType.Copy)
                xtb_t = const.tile([128, NCH * POS], BF16)   # chunk g at g*POS
                nc.sync.dma_start(
                    xtb_t[:].rearrange("p (g m) -> p g m", g=NCH),
                    xT[:].rearrange("(g p) m -> p g m", g=NCH))
                wd_t = const.tile([128, NCH * NF * 32], BF16)
                nc.sync.dma_start(
                    wd_t[:].rearrange("p (g m) -> p g m", g=NCH),
                    wd[:].rearrange("(g p) m -> p g m", g=NCH))
                bdw_t = const.tile([128, NF * NCH * 256], BF16)
                for f in range(NF):
                    nc.sync.dma_start(bdw_t[:, f * 2048:(f + 1) * 2048],
                                      bdw[:, f * 2048:(f + 1) * 2048])

                with tc.tile_pool(name="ubp", bufs=2) as ubp, \
                     tc.tile_pool(name="big", bufs=1) as big, \
                     tc.tile_pool(name="psum_u", bufs=2, space="PSUM") as psum_u, \
                     tc.tile_pool(name="psum_s", bufs=2, space="PSUM") as psum_s:

                    def _prep(b):
                        """s0+squash then u_hat for block b -> (vb16, ub).

                        Called between it0 and it1 of block b-1's routing so
                        the five ACT drains land while DVE is busy, instead
                        of compressing into the block boundary (ACT is
                        in-order)."""
                        ps = psum_s.tile([BLK, 160], F32, tag="ps")
                        for f in range(NF):
                            for g in range(NCH):
                                nc.tensor.matmul(
                                    ps[:, f * 32:(f + 1) * 32],
                                    xtb_t[:, g * POS + b * BLK: g * POS + (b + 1) * BLK],
                                    wd_t[:, g * 160 + f * 32: g * 160 + (f + 1) * 32],
                                    start=(g == 0), stop=(g == NCH - 1))
                        s0b = work.tile([BLK, 160], F32, tag="s0b")
                        nc.scalar.activation(s0b[:], ps[:],
                                             mybir.ActivationFunctionType.Copy,
                                             scale=1.0 / N_IN)
                        vb16 = work.tile([BLK, 160], BF16, tag="vb16")
                        _squash(nc, work, s0b[:], vb16[:])
                        ub = ubp.tile([BLK, NF * 2048], BF16, tag="ub")
                        for f in range(NF):
                            for h in range(2):  # half-PSUM tiles, bufs=2
                                up = psum_u.tile([BLK, 1024], F32, tag="up")
                                for g in range(4 * h, 4 * h + 4):
                                    nc.tensor.matmul(
                                        up[:, (g % 4) * 256:(g % 4 + 1) * 256],
                                        xtb_t[:, g * POS + b * BLK: g * POS + (b + 1) * BLK],
                                        bdw_t[:, f * 2048 + g * 256: f * 2048 + (g + 1) * 256],
                                        start=True, stop=True)
                                nc.scalar.activation(
                                    ub[:, f * 2048 + h * 1024:
                                       f * 2048 + (h + 1) * 1024],
                                    up[:],
                                    mybir.ActivationFunctionType.Copy)
                        return vb16, ub

                    state = _prep(0)
                    for b in range(NBLK):
                        vb16, ub = state
                        bco = work.tile([BLK, NF * 128], BF16, tag="bco")
                        for it in range(2):
                            if it == 1 and b + 1 < NBLK:
                                state = _prep(b + 1)
                            # --- vE: v expanded over i8 (per f) ---
                            vE = big.tile([BLK, NF * 256], BF16, tag="vE")
                            for f in range(NF):
                                nc.vector.tensor_copy(
                                    vE[:, f * 256:(f + 1) * 256].rearrange(
                                        "p (o i n) -> p o i n", o=16, i=8),
                                    vb16[:, f * 32:(f + 1) * 32].rearrange(
                                        "p (o n) -> p o n", o=16)
                                    .unsqueeze(2).broadcast_to([BLK, 16, 8, 2]))
                            # --- P = U * vE (broadcast over g) ---
                            P = big.tile([BLK, NF * 2048], BF16, tag="P")
                            for f in range(NF):
                                nc.vector.tensor_mul(
                                    P[:, f * 2048:(f + 1) * 2048].rearrange(
                                        "p (g x) -> p g x", g=8),
                                    ub[:, f * 2048:(f + 1) * 2048].rearrange(
                                        "p (g x) -> p g x", g=8),
                                    vE[:, f * 256:(f + 1) * 256]
                                    .unsqueeze(1).broadcast_to([BLK, 8, 256]))
                            # --- agreement: agr = sum_o P (o-halving tree) ---
                            with nc.allow_low_precision("bf16 tree sums"):
                                Pv = P[:].rearrange("p (s o x) -> p s o x",
                                                    o=16, x=16)
                                t1 = big.tile([BLK, NF * 1024], BF16, tag="t1")
                                t1v = t1[:].rearrange("p (s o x) -> p s o x",
                                                      o=8, x=16)
                                nc.vector.tensor_add(t1v, Pv[:, :, 0:8],
                                                     Pv[:, :, 8:16])
                                t2 = big.tile([BLK, NF * 512], BF16, tag="t2")
                                t2v = t2[:].rearrange("p (s o x) -> p s o x",
                                                      o=4, x=16)
                                nc.vector.tensor_add(t2v, t1v[:, :, 0:4],
                                                     t1v[:, :, 4:8])
                                t3 = big.tile([BLK, NF * 256], BF16, tag="t3")
                                t3v = t3[:].rearrange("p (s o x) -> p s o x",
                                                      o=2, x=16)
                                nc.vector.tensor_add(t3v, t2v[:, :, 0:2],
                                                     t2v[:, :, 2:4])
                                agr = work.tile([BLK, NF * 128], BF16,
                                                tag="agr")
                                nc.vector.tensor_add(
                                    agr[:].rearrange("p (s o x) -> p s o x",
                                                     o=1, x=16),
                                    t3v[:, :, 0:1], t3v[:, :, 1:2])
                                # --- b update ---
                                if it == 0:
                                    nc.vector.tensor_scalar_add(
                                        bco[:], agr[:], 1.0 / N_IN)
                                else:
                                    nc.vector.tensor_add(bco[:], bco[:],
                                                         agr[:])
                            # --- Q = b * U (broadcast b over o) ---
                            Q = big.tile([BLK, NF * 2048], BF16, tag="Q")
                            for f in range(NF):
                                bf = bco[:, f * 128:(f + 1) * 128] \
                                    .rearrange("p (g x) -> p g x", g=8) \
                                    .unsqueeze(2).broadcast_to([BLK, 8, 16, 16])
                                nc.vector.tensor_mul(
                                    Q[:, f * 2048:(f + 1) * 2048].rearrange(
                                        "p (g o x) -> p g o x", g=8, o=16),
                                    ub[:, f * 2048:(f + 1) * 2048].rearrange(
                                        "p (g o x) -> p g o x", g=8, o=16),
                                    bf)
                            # --- v-sum: s = sum_{g,i8} Q (halving trees) ---
                            with nc.allow_low_precision("bf16 tree sums"):
                                Qv = Q[:].rearrange("p (s i n) -> p s i n",
                                                    i=8, n=2)
                                u1 = big.tile([BLK, NF * 1024], BF16, tag="u1")
                                u1v = u1[:].rearrange("p (s i n) -> p s i n",
                                                      i=4, n=2)
                                nc.vector.tensor_add(u1v, Qv[:, :, 0:4],
                                                     Qv[:, :, 4:8])
                                u2 = big.tile([BLK, NF * 512], BF16, tag="u2")
                                u2v = u2[:].rearrange("p (s i n) -> p s i n",
                                                      i=2, n=2)
                                nc.vector.tensor_add(u2v, u1v[:, :, 0:2],
                                                     u1v[:, :, 2:4])
                                u3 = big.tile([BLK, NF * 256], BF16, tag="u3")
                                u3v = u3[:].rearrange("p (s i n) -> p s i n",
                                                      i=1, n=2)
                                nc.vector.tensor_add(u3v, u2v[:, :, 0:1],
                                                     u2v[:, :, 1:2])
                                # u3 layout (f, g, o, n2): sum over g
                                u3g = u3[:].rearrange("p (f g y) -> p f g y",
                                                      f=NF, g=8)
                                u4 = big.tile([BLK, NF * 128], BF16, tag="u4")
                                u4v = u4[:].rearrange("p (f g y) -> p f g y",
                                                      f=NF, g=4)
                                nc.vector.tensor_add(u4v, u3g[:, :, 0:4],
                                                     u3g[:, :, 4:8])
                                u5 = big.tile([BLK, NF * 64], BF16, tag="u5")
                                u5v = u5[:].rearrange("p (f g y) -> p f g y",
                                                      f=NF, g=2)
                                nc.vector.tensor_add(u5v, u4v[:, :, 0:2],
                                                     u4v[:, :, 2:4])
                                s_blk = work.tile([BLK, 160], F32, tag="s_blk")
                                nc.vector.tensor_add(
                                    s_blk[:].rearrange("p (f g y) -> p f g y",
                                                       f=NF, g=1),
                                    u5v[:, :, 0:1], u5v[:, :, 1:2])
                            # --- squash ---
                            if it == 0:
                                _squash(nc, work, s_blk[:], vb16[:])
                            else:
                                outb = work.tile([BLK, 160], F32, tag="outb")
                                _squash(nc, work, s_blk[:], outb[:])
                                nc.sync.dma_start(
                                    out[b * BLK:(b + 1) * BLK, :], outb[:])
    nc.compile()
    return nc


def _host_prep(inputs, W):
    """Build per-core input maps from full inputs."""
    import ml_dtypes
    x = np.ascontiguousarray(inputs, dtype=np.float32).reshape(B, R * C, IE)
    Wf = np.ascontiguousarray(W, dtype=np.float32)  # [n, i, e, o]
    # bdw[(i8,e), (f, g, o, i8', n2)] block-diagonal over i8
    Wg = Wf.reshape(N_CAPS, 8, 8, D_IN, CAPS_DIM)   # [n, g, i8, e, o]
    bdw7 = np.zeros((8, D_IN, NF, NCH, CAPS_DIM, 8, 2), dtype=np.float32)
    for i8 in range(8):
        # Wg[:, :, i8] : [n, g, e, o] -> [f, n2, g, e, o] -> [e, f, g, o, n2]
        w5 = Wg[:, :, i8, :, :].reshape(NF, 2, NCH, D_IN, CAPS_DIM)
        bdw7[i8, :, :, :, :, i8, :] = w5.transpose(3, 0, 2, 4, 1)
    bdw = bdw7.reshape(128, NCH * NF * 256).astype(ml_dtypes.bfloat16)
    # wd[(i,e), (f, o, n2)]
    w5d = Wf.reshape(NF, 2, N_IN, D_IN, CAPS_DIM)   # [f, n2, i, e, o]
    wd = np.ascontiguousarray(
        w5d.transpose(2, 3, 0, 4, 1).reshape(IE, NF * 32)
    ).astype(ml_dtypes.bfloat16)
    bpc = B // N_CORES
    in_maps = []
    for c in range(N_CORES):
        xc = x[c * bpc:(c + 1) * bpc].reshape(POS, IE)
        in_maps.append({
            "xT": np.ascontiguousarray(xc.T).astype(ml_dtypes.bfloat16),
            "bdw": bdw,
            "wd": wd,
        })
    return in_maps


_NC_CACHE = []


def kernel(inputs: np.ndarray, W: np.ndarray) -> np.ndarray:
    in_maps = _host_prep(inputs, W)
    if not _NC_CACHE:
        _NC_CACHE.append(build_kernel())
    nc = _NC_CACHE[0]
    res = run_bass_kernel_spmd(nc, in_maps, list(range(N_CORES)))
    outs = [res.results[c]["out"] for c in range(N_CORES)]
    full = np.concatenate(outs, axis=0)  # [3136, (f, o, n2)]
    # un-permute columns (f, o, n2) -> (n, o) with n = 2f + n2
    full = full.reshape(B * R * C, NF, CAPS_DIM, 2).transpose(0, 1, 3, 2)
    return np.ascontiguousarray(
        full.reshape(B, R, C, N_CAPS, CAPS_DIM))


# revision 23
# speedup vs baseline: 2.1933x; 2.1933x over previous
"""CapsLayer2D Trainium2 kernel (8-core SPMD, data-parallel over batch).

Math: per position p (of B*R*C) and capsule n:
  U[n,i,o] = sum_e x[p,i,e] * W[n,i,e,o]          (u_hat)
  b0 = 1/64; 2x { v = squash(sum_i b*U); b += sum_o U*v }; out = squash(sum_i b*U)

Mapping:
  - 8 cores, 2 batches each -> 392 positions/core, processed as 2
    superblocks x 2 sub-blocks of 98 positions (partition dim).
  - Per superblock: u_hat via block-diagonal-W bf16 matmuls into PSUM,
    ACT-drained to bf16 SBUF; s0 = sum_i U/64 via dense K=1024 bf16 matmuls
    (v0 = squash(s0), exact since b0 is uniform); then 2 routing iterations
    on DVE over both sub-blocks at once (halves per-op overhead).
  - Free-dim layout is (sub, f, g, o, x) [sub-block 2, n-pair f=5, K-chunk
    g=8, caps_dim o=16, x=(i8,n2)=16] so every DVE tensor_tensor has an
    innermost stride-1 bf16 run (2x_1P perf mode) within <=3 free AP dims
    (TENSOR3D ISA limit). k below indexes (sub, f) = 10 units.
  - Superblock bb+1's prep (s0, squash, u_hat + drains) is issued between
    it0 and it1 of superblock bb so the ACT drains overlap routing (ACT is
    in-order).
  - P/Q and the two trees' temporaries share buffers (disjoint lifetimes).
  - Host pre-builds bf16 xT, BD(W), dense W with matching column orders and
    un-permutes the (f,o,n2) output columns back to (n,o).
"""
import numpy as np

import concourse.bacc as bacc
import concourse.bass as bass
import concourse.mybir as mybir
import concourse.tile as tile
from concourse.bass_utils import run_bass_kernel_spmd

N_CORES = 8
B, R, C = 16, 14, 14
N_IN, D_IN = 64, 16          # i, e
N_CAPS, CAPS_DIM = 10, 16    # n, o
IE = N_IN * D_IN             # 1024
POS = (B // N_CORES) * R * C # 392 positions per core
BLK = 98                     # sub-block size (partition dim)
NBLK = POS // BLK            # 4 sub-blocks
SB = 2                       # superblocks of 2 sub-blocks
NF = N_CAPS // 2             # 5 units of 2 capsules
NK = 2 * NF                  # (sub, f) units per superblock
NCH = IE // 128              # 8 contraction chunks
F32 = mybir.dt.float32
BF16 = mybir.dt.bfloat16


def _squash(nc, pool, s_ap, v_ap):
    """v = squash(s). s_ap f32 [98,320]=(k=10,o,n2) contiguous; v_ap same
    layout (any dtype). Square runs on DVE (s*s) to avoid an ACT
    round-trip on the critical path; Sqrt stays on ACT."""
    P = s_ap.shape[0]
    sq = pool.tile([P, NK * 32], F32, tag="sq")
    nc.vector.tensor_mul(sq[:], s_ap, s_ap)
    q = pool.tile([P, NK * 2], F32, tag="q")
    # reduce over o: view (k, n2, o) with o innermost (stride 2)
    nc.vector.tensor_reduce(
        q[:].rearrange("p (k n) -> p k n", k=NK),
        sq[:].rearrange("p (k o n) -> p k n o", k=NK, o=16),
        axis=mybir.AxisListType.X, op=mybir.AluOpType.add)
    rt = pool.tile([P, NK * 2], F32, tag="rt")
    nc.scalar.activation(rt[:], q[:], mybir.ActivationFunctionType.Sqrt)
    qp = pool.tile([P, NK * 2], F32, tag="qp")
    nc.vector.tensor_scalar_add(qp[:], q[:], 1.0)
    rc = pool.tile([P, NK * 2], F32, tag="rc")
    nc.vector.reciprocal(rc[:], qp[:])
    al = pool.tile([P, NK * 2], F32, tag="al")
    nc.vector.tensor_mul(al[:], rt[:], rc[:])
    alb = al[:].rearrange("p (k n) -> p k n", k=NK) \
        .unsqueeze(2).broadcast_to([P, NK, 16, 2])
    nc.vector.tensor_mul(
        v_ap.rearrange("p (k o n) -> p k o n", k=NK, o=16),
        s_ap.rearrange("p (k o n) -> p k o n", k=NK, o=16), alb)


def build_kernel(dbg=False, repeat=1):
    nc = bacc.Bacc("TRN2", target_bir_lowering=False, debug=False,
                   num_devices=N_CORES)
    xT = nc.dram_tensor("xT", [IE, POS], BF16, kind="ExternalInput").ap()
    bdw = nc.dram_tensor("bdw", [128, NCH * NF * 256], BF16,
                         kind="ExternalInput").ap()
    wd = nc.dram_tensor("wd", [IE, NF * 32], BF16, kind="ExternalInput").ap()
    out = nc.dram_tensor("out", [POS, NF * 32], F32,
                         kind="ExternalOutput").ap()

    with tile.TileContext(nc) as tc:
        for _rep in range(repeat):
            with tc.tile_pool(name="const", bufs=1) as const, \
                 tc.tile_pool(name="work", bufs=2) as work, \
                 tc.tile_pool(name="sqp", bufs=1) as sqp:
                # warm the ACT tables (Sqrt/Copy) before the big DMAs so the
                # table-load DMAs don't queue behind them
                warm = const.tile([1, 2], F32)
                nc.vector.memset(warm[:], 1.0)
                nc.scalar.activation(warm[:], warm[:],
                                     mybir.ActivationFunctionType.Sqrt)
                nc.scalar.activation(warm[:], warm[:],
                                     mybir.ActivationFunctionType.Copy)
                # split the input DMAs over two HWDGE queues (SP + ACT) so
                # the xT halves land in parallel; bdw f0 ahead of f1-4 since
                # superblock 0's first u_hat needs it
                xtb_t = const.tile([128, NCH * POS], BF16)   # chunk g at g*POS
                H = NCH // 2
                nc.sync.dma_start(
                    xtb_t[:, :H * POS].rearrange("p (g m) -> p g m", g=H),
                    xT[:H * 128, :].rearrange("(g p) m -> p g m", g=H))
                nc.scalar.dma_start(
                    xtb_t[:, H * POS:].rearrange("p (g m) -> p g m", g=H),
                    xT[H * 128:, :].rearrange("(g p) m -> p g m", g=H))
                wd_t = const.tile([128, NCH * NF * 32], BF16)
                nc.sync.dma_start(
                    wd_t[:].rearrange("p (g m) -> p g m", g=NCH),
                    wd[:].rearrange("(g p) m -> p g m", g=NCH))
                bdw_t = const.tile([128, NF * NCH * 256], BF16)
                nc.scalar.dma_start(bdw_t[:, 0:2048], bdw[:, 0:2048])
                for f in range(1, NF):
                    nc.sync.dma_start(bdw_t[:, f * 2048:(f + 1) * 2048],
                                      bdw[:, f * 2048:(f + 1) * 2048])

                with tc.tile_pool(name="ubp", bufs=2) as ubp, \
                     tc.tile_pool(name="big", bufs=1) as big, \
                     tc.tile_pool(name="psum_u", bufs=2, space="PSUM") as psum_u, \
                     tc.tile_pool(name="psum_s", bufs=2, space="PSUM") as psum_s:

                    def _prep(bb):
                        """s0+squash then u_hat for superblock bb."""
                        ps = psum_s.tile([BLK, NK * 32], F32, tag="ps")
                        for sub in range(2):
                            blk = bb * 2 + sub
                            for f in range(NF):
                                for g in range(NCH):
                                    nc.tensor.matmul(
                                        ps[:, sub * 160 + f * 32:
                                           sub * 160 + (f + 1) * 32],
                                        xtb_t[:, g * POS + blk * BLK:
                                              g * POS + (blk + 1) * BLK],
                                        wd_t[:, g * 160 + f * 32:
                                             g * 160 + (f + 1) * 32],
                                        start=(g == 0), stop=(g == NCH - 1))
                        s0b = work.tile([BLK, NK * 32], F32, tag="s0b")
                        nc.scalar.activation(s0b[:], ps[:],
                                             mybir.ActivationFunctionType.Copy,
                                             scale=1.0 / N_IN)
                        vb16 = work.tile([BLK, NK * 32], BF16, tag="vb16")
                        _squash(nc, sqp, s0b[:], vb16[:])
                        ub = ubp.tile([BLK, NK * 2048], BF16, tag="ub")
                        for sub in range(2):
                            blk = bb * 2 + sub
                            for f in range(NF):
                                for h in range(2):  # half-PSUM tiles, bufs=2
                                    up = psum_u.tile([BLK, 1024], F32,
                                                     tag="up")
                                    for g in range(4 * h, 4 * h + 4):
                                        nc.tensor.matmul(
                                            up[:, (g % 4) * 256:
                                               (g % 4 + 1) * 256],
                                            xtb_t[:, g * POS + blk * BLK:
                                                  g * POS + (blk + 1) * BLK],
                                            bdw_t[:, f * 2048 + g * 256:
                                                  f * 2048 + (g + 1) * 256],
                                            start=True, stop=True)
                                    nc.scalar.activation(
                                        ub[:, sub * 10240 + f * 2048 +
                                           h * 1024:
                                           sub * 10240 + f * 2048 +
                                           (h + 1) * 1024],
                                        up[:],
                                        mybir.ActivationFunctionType.Copy)
                        return vb16, ub

                    state = _prep(0)
                    for bb in range(SB):
                        vb16, ub = state
                        bco = work.tile([BLK, NK * 128], BF16, tag="bco")
                        for it in range(2):
                            if it == 1 and bb + 1 < SB:
                                state = _prep(bb + 1)
                            # --- vE: v expanded over i8 (per k) ---
                            # (shares the T2 buffer; disjoint lifetime)
                            vE_full = big.tile([BLK, NK * 512], BF16,
                                               tag="T2")
                            vE = vE_full[:, :NK * 256]
                            for k in range(NK):
                                nc.vector.tensor_copy(
                                    vE[:, k * 256:(k + 1) * 256].rearrange(
                                        "p (o i n) -> p o i n", o=16, i=8),
                                    vb16[:, k * 32:(k + 1) * 32].rearrange(
                                        "p (o n) -> p o n", o=16)
                                    .unsqueeze(2).broadcast_to([BLK, 16, 8, 2]))
                            # --- P = U * vE (broadcast over g) ---
                            P = big.tile([BLK, NK * 2048], BF16, tag="PQ")
                            if bb == 0 and it == 0:
                                # pipeline fill: per-(k, psum-half) so each
                                # P-mul waits only on its own u_hat drain
                                for k in range(NK):
                                    for h in range(2):
                                        o0 = k * 2048 + h * 1024
                                        nc.vector.tensor_mul(
                                            P[:, o0:o0 + 1024].rearrange(
                                                "p (g x) -> p g x", g=4),
                                            ub[:, o0:o0 + 1024].rearrange(
                                                "p (g x) -> p g x", g=4),
                                            vE[:, k * 256:(k + 1) * 256]
                                            .unsqueeze(1).broadcast_to(
                                                [BLK, 4, 256]))
                            else:
                                nc.vector.tensor_mul(
                                    P[:].rearrange("p (k g x) -> p k g x",
                                                   k=NK, g=8),
                                    ub[:].rearrange("p (k g x) -> p k g x",
                                                    k=NK, g=8),
                                    vE.rearrange("p (k x) -> p k x", k=NK)
                                    .unsqueeze(2).broadcast_to(
                                        [BLK, NK, 8, 256]))
                            # --- agreement: agr = sum_o P (o-halving tree) ---
                            with nc.allow_low_precision("bf16 tree sums"):
                                Pv = P[:].rearrange("p (s o x) -> p s o x",
                                                    o=16, x=16)
                                t1 = big.tile([BLK, NK * 1024], BF16,
                                              tag="T1")
                                t1v = t1[:].rearrange("p (s o x) -> p s o x",
                                                      o=8, x=16)
                                nc.vector.tensor_add(t1v, Pv[:, :, 0:8],
                                                     Pv[:, :, 8:16])
                                t2 = big.tile([BLK, NK * 512], BF16,
                                              tag="T2")
                                t2v = t2[:].rearrange("p (s o x) -> p s o x",
                                                      o=4, x=16)
                                nc.vector.tensor_add(t2v, t1v[:, :, 0:4],
                                                     t1v[:, :, 4:8])
                                t3 = big.tile([BLK, NK * 256], BF16,
                                              tag="T3")
                                t3v = t3[:].rearrange("p (s o x) -> p s o x",
                                                      o=2, x=16)
                                nc.vector.tensor_add(t3v, t2v[:, :, 0:2],
                                                     t2v[:, :, 2:4])
                                agr = big.tile([BLK, NK * 128], BF16,
                                               tag="agr")
                                nc.vector.tensor_add(
                                    agr[:].rearrange("p (s o x) -> p s o x",
                                                     o=1, x=16),
                                    t3v[:, :, 0:1], t3v[:, :, 1:2])
                                # --- b update ---
                                if it == 0:
                                    nc.vector.tensor_scalar_add(
                                        bco[:], agr[:], 1.0 / N_IN)
                                else:
                                    nc.vector.tensor_add(bco[:], bco[:],
                                                         agr[:])
                            # --- Q = b * U (broadcast b over o) ---
                            Q = big.tile([BLK, NK * 2048], BF16, tag="PQ")
                            for k in range(NK):
                                bf = bco[:, k * 128:(k + 1) * 128] \
                                    .rearrange("p (g x) -> p g x", g=8) \
                                    .unsqueeze(2).broadcast_to([BLK, 8, 16, 16])
                                nc.vector.tensor_mul(
                                    Q[:, k * 2048:(k + 1) * 2048].rearrange(
                                        "p (g o x) -> p g o x", g=8, o=16),
                                    ub[:, k * 2048:(k + 1) * 2048].rearrange(
                                        "p (g o x) -> p g o x", g=8, o=16),
                                    bf)
                            # --- v-sum: s = sum_{g,i8} Q (halving trees) ---
                            with nc.allow_low_precision("bf16 tree sums"):
                                Qv = Q[:].rearrange("p (s i n) -> p s i n",
                                                    i=8, n=2)
                                u1 = big.tile([BLK, NK * 1024], BF16,
                                              tag="T1")
                                u1v = u1[:].rearrange("p (s i n) -> p s i n",
                                                      i=4, n=2)
                                nc.vector.tensor_add(u1v, Qv[:, :, 0:4],
                                                     Qv[:, :, 4:8])
                                u2 = big.tile([BLK, NK * 512], BF16,
                                              tag="T2")
                                u2v = u2[:].rearrange("p (s i n) -> p s i n",
                                                      i=2, n=2)
                                nc.vector.tensor_add(u2v, u1v[:, :, 0:2],
                                                     u1v[:, :, 2:4])
                                u3 = big.tile([BLK, NK * 256], BF16,
                                              tag="T3")
                                u3v = u3[:].rearrange("p (s i n) -> p s i n",
                                                      i=1, n=2)
                                nc.vector.tensor_add(u3v, u2v[:, :, 0:1],
                                                     u2v[:, :, 1:2])
                                # u3 layout (k, g, o, n2): sum over g
                                u3g = u3[:].rearrange("p (k g y) -> p k g y",
                                                      k=NK, g=8)
                                u4 = big.tile([BLK, NK * 128], BF16,
                                              tag="u4")
                                u4v = u4[:].rearrange("p (k g y) -> p k g y",
                                                      k=NK, g=4)
                                nc.vector.tensor_add(u4v, u3g[:, :, 0:4],
                                                     u3g[:, :, 4:8])
                                u5 = big.tile([BLK, NK * 64], BF16,
                                              tag="u5")
                                u5v = u5[:].rearrange("p (k g y) -> p k g y",
                                                      k=NK, g=2)
                                nc.vector.tensor_add(u5v, u4v[:, :, 0:2],
                                                     u4v[:, :, 2:4])
                                s_blk = sqp.tile([BLK, NK * 32], F32,
                                                 tag="s_blk")
                                nc.vector.tensor_add(
                                    s_blk[:].rearrange("p (k g y) -> p k g y",
                                                       k=NK, g=1),
                                    u5v[:, :, 0:1], u5v[:, :, 1:2])
                            # --- squash ---
                            if it == 0:
                                _squash(nc, sqp, s_blk[:], vb16[:])
                            else:
                                outb = work.tile([BLK, NK * 32], F32,
                                                 tag="outb")
                                _squash(nc, sqp, s_blk[:], outb[:])
                                nc.sync.dma_start(
                                    out[bb * 2 * BLK:(bb + 1) * 2 * BLK, :]
                                    .rearrange("(s p) m -> p s m", s=2),
                                    outb[:].rearrange("p (s m) -> p s m",
                                                      s=2))
    nc.compile()
    return nc


def _host_prep(inputs, W):
    """Build per-core input maps from full inputs."""
    import ml_dtypes
    x = np.ascontiguousarray(inputs, dtype=np.float32).reshape(B, R * C, IE)
    Wf = np.ascontiguousarray(W, dtype=np.float32)  # [n, i, e, o]
    # bdw[(i8,e), (f, g, o, i8', n2)] block-diagonal over i8
    Wg = Wf.reshape(N_CAPS, 8, 8, D_IN, CAPS_DIM)   # [n, g, i8, e, o]
    bdw7 = np.zeros((8, D_IN, NF, NCH, CAPS_DIM, 8, 2), dtype=np.float32)
    for i8 in range(8):
        # Wg[:, :, i8] : [n, g, e, o] -> [f, n2, g, e, o] -> [e, f, g, o, n2]
        w5 = Wg[:, :, i8, :, :].reshape(NF, 2, NCH, D_IN, CAPS_DIM)
        bdw7[i8, :, :, :, :, i8, :] = w5.transpose(3, 0, 2, 4, 1)
    bdw = bdw7.reshape(128, NCH * NF * 256).astype(ml_dtypes.bfloat16)
    # wd[(i,e), (f, o, n2)]
    w5d = Wf.reshape(NF, 2, N_IN, D_IN, CAPS_DIM)   # [f, n2, i, e, o]
    wd = np.ascontiguousarray(
        w5d.transpose(2, 3, 0, 4, 1).reshape(IE, NF * 32)
    ).astype(ml_dtypes.bfloat16)
    bpc = B // N_CORES
    in_maps = []
    for c in range(N_CORES):
        xc = x[c * bpc:(c + 1) * bpc].reshape(POS, IE)
        in_maps.append({
            "xT": np.ascontiguousarray(xc.T).astype(ml_dtypes.bfloat16),
            "bdw": bdw,
            "wd": wd,
        })
    return in_maps


_NC_CACHE = []


def kernel(inputs: np.ndarray, W: np.ndarray) -> np.ndarray:
    in_maps = _host_prep(inputs, W)
    if not _NC_CACHE:
        _NC_CACHE.append(build_kernel())
    nc = _NC_CACHE[0]
    res = run_bass_kernel_spmd(nc, in_maps, list(range(N_CORES)))
    outs = [res.results[c]["out"] for c in range(N_CORES)]
    full = np.concatenate(outs, axis=0)  # [3136, (f, o, n2)]
    # un-permute columns (f, o, n2) -> (n, o) with n = 2f + n2
    full = full.reshape(B * R * C, NF, CAPS_DIM, 2).transpose(0, 1, 3, 2)
    return np.ascontiguousarray(
        full.reshape(B, R, C, N_CAPS, CAPS_DIM))
